# revision 1
# baseline (speedup 1.0000x reference)
"""Trainium2 Bass kernel for KVAdapterInjector (Qwen3-style GQA attention with
LoRA-adapted virtual KV prefix).

Sharding: tensor-parallel over heads across 8 cores. Core m gets KV head m and
Q heads 4m..4m+3. Wq/Wk/Wv sharded on output dim, Wo on input dim; partial
outputs summed on host.

All heavy matmuls run fp32r (full PE rate at N>=512). Layouts keep the
contraction dim on partitions everywhere, so no on-device transposes of
hidden_states are needed (host passes hs^T, cos^T, sin^T).
"""
import sys

sys.path.insert(0, "/opt/trn_rl_repo")

import numpy as np

import concourse.bass as bass
import concourse.mybir as mybir
import concourse.tile as tile
from concourse import bacc
from concourse.bass_utils import run_bass_kernel_spmd

F32 = mybir.dt.float32
F32R = mybir.dt.float32r
AX = mybir.AxisListType
ALU = mybir.AluOpType
ACTF = mybir.ActivationFunctionType

T = 2048
D = 4096
HD = 128
NQH = 4          # q heads per core
R = 64           # virtual tokens
RANK = 16
EPS = 1e-6
SCALING = HD ** -0.5
NTC = 4          # T chunks of 512
TC = 512
ND = D // 128    # 32 contraction tiles


def build_nc():
    nc = bacc.Bacc(None, target_bir_lowering=False, debug=False)

    # ---- DRAM I/O ----
    hsT = nc.dram_tensor("hsT", (D, T), F32, kind="ExternalInput")
    wq = nc.dram_tensor("wq", (D, NQH * HD), F32, kind="ExternalInput")
    wk = nc.dram_tensor("wk", (D, HD), F32, kind="ExternalInput")
    wv = nc.dram_tensor("wv", (D, HD), F32, kind="ExternalInput")
    wo = nc.dram_tensor("wo", (NQH * HD, D), F32, kind="ExternalInput")
    vkT = nc.dram_tensor("vkT", (HD, R), F32, kind="ExternalInput")
    vvT = nc.dram_tensor("vvT", (HD, R), F32, kind="ExternalInput")
    lkA = nc.dram_tensor("lkA", (HD, RANK), F32, kind="ExternalInput")
    lkB = nc.dram_tensor("lkB", (RANK, HD), F32, kind="ExternalInput")  # pre-scaled
    lvA = nc.dram_tensor("lvA", (HD, RANK), F32, kind="ExternalInput")
    lvB = nc.dram_tensor("lvB", (RANK, HD), F32, kind="ExternalInput")  # pre-scaled
    qw = nc.dram_tensor("qw", (HD, 1), F32, kind="ExternalInput")
    kw = nc.dram_tensor("kw", (HD, 1), F32, kind="ExternalInput")
    cosT = nc.dram_tensor("cosT", (HD, T), F32, kind="ExternalInput")
    sinT = nc.dram_tensor("sinT", (HD, T), F32, kind="ExternalInput")
    maskT = nc.dram_tensor("maskT", (128, 4 * TC), F32, kind="ExternalInput")
    rotm = nc.dram_tensor("rotm", (HD, HD), F32, kind="ExternalInput")
    ident = nc.dram_tensor("ident", (128, 128), F32, kind="ExternalInput")
    onesc = nc.dram_tensor("onesc", (128, 1), F32, kind="ExternalInput")
    onesr = nc.dram_tensor("onesr", (1, 128), F32, kind="ExternalInput")
    out = nc.dram_tensor("out", (T, D), F32, kind="ExternalOutput")

    r = lambda ap: ap.bitcast(F32R)

    from contextlib import ExitStack
    with tile.TileContext(nc) as tc, ExitStack() as est:
        cp = est.enter_context(tc.tile_pool(name="consts", bufs=1))
        pp = est.enter_context(tc.tile_pool(name="persist", bufs=1))

        # ---- consts in SBUF ----
        cosT_s = cp.tile([HD, T], F32)
        sinT_s = cp.tile([HD, T], F32)
        maskT_s = cp.tile([128, 4 * TC], F32)
        rotm_s = cp.tile([HD, HD], F32R)
        ident_s = cp.tile([128, 128], F32R)
        onesc_s = cp.tile([128, 1], F32R)
        onesr_s = cp.tile([1, 128], F32R)
        qw_s = cp.tile([HD, 1], F32)
        epsc = cp.tile([128, 1], F32)
        nc.vector.memset(epsc[:], EPS)
        kw_s = cp.tile([HD, 1], F32)
        vkT_s = cp.tile([HD, R], F32R)
        vvT_s = cp.tile([HD, R], F32R)
        lkA_s = cp.tile([HD, RANK], F32R)
        lkB_s = cp.tile([RANK, HD], F32R)
        lvA_s = cp.tile([HD, RANK], F32R)
        lvB_s = cp.tile([RANK, HD], F32R)
        nc.sync.dma_start(cosT_s[:], cosT[:])
        nc.sync.dma_start(sinT_s[:], sinT[:])
        nc.sync.dma_start(maskT_s[:], maskT[:])
        nc.sync.dma_start(rotm_s[:], r(rotm[:]))
        nc.sync.dma_start(ident_s[:], r(ident[:]))
        nc.sync.dma_start(onesc_s[:], r(onesc[:]))
        nc.sync.dma_start(onesr_s[:], r(onesr[:]))
        nc.sync.dma_start(qw_s[:], qw[:])
        nc.sync.dma_start(kw_s[:], kw[:])
        nc.sync.dma_start(vkT_s[:], r(vkT[:]))
        nc.sync.dma_start(vvT_s[:], r(vvT[:]))
        nc.sync.dma_start(lkA_s[:], r(lkA[:]))
        nc.sync.dma_start(lkB_s[:], r(lkB[:]))
        nc.sync.dma_start(lvA_s[:], r(lvA[:]))
        nc.sync.dma_start(lvB_s[:], r(lvB[:]))

        # ---- persistent activations ----
        qT = [pp.tile([HD, T], F32R, tag=f"qT{h}", name=f"qT{h}") for h in range(NQH)]
        kT = pp.tile([HD, R + T], F32R)           # cols 0:64 = adapted virtual keys
        vT = pp.tile([HD, T], F32R)
        vnat = pp.tile([128, 128 + T], F32R)      # slot b at cols 128b; slot 0 virtual
        oT = qT  # alias: qT[h][:, ts] is dead after its attention chunk
        vvirtT = pp.tile([HD, R], F32R)

        # ================= Phase 1: LoRA-adapt virtual KV =================
        with tc.tile_pool(name="lora_ps", bufs=1, space="PSUM") as lps, \
             tc.tile_pool(name="lora_sb", bufs=4) as lsb:
            # keys: kT[:, 0:64] = vkT + Bk^T Ak^T vkT  (Bk pre-scaled)
            t1 = lps.tile([RANK, R], F32)
            nc.tensor.matmul(t1[:], lkA_s[:], vkT_s[:], start=True, stop=True)
            t1s = lsb.tile([RANK, R], F32R)
            nc.scalar.copy(t1s[:], t1[:])
            t2 = lps.tile([HD, R], F32)
            nc.tensor.matmul(t2[:], lkB_s[:], t1s[:], start=True, stop=True)
            nc.vector.tensor_add(kT[:, 0:R], vkT_s[:].bitcast(F32), t2[:])
            # values
            u1 = lps.tile([RANK, R], F32)
            nc.tensor.matmul(u1[:], lvA_s[:], vvT_s[:], start=True, stop=True)
            u1s = lsb.tile([RANK, R], F32R)
            nc.scalar.copy(u1s[:], u1[:])
            u2 = lps.tile([HD, R], F32)
            nc.tensor.matmul(u2[:], lvB_s[:], u1s[:], start=True, stop=True)
            nc.vector.tensor_add(vvirtT[:], vvT_s[:].bitcast(F32), u2[:])
            # transpose virtual values to natural layout -> vnat[0:64, 0:128]
            vtp = lps.tile([R, HD], F32R)
            nc.tensor.transpose(vtp[:], vvirtT[:], ident_s[:])
            nc.scalar.copy(vnat[0:R, 0:128], vtp[:])

        # ================= Phase 2: QKV projections =================
        with tc.tile_pool(name="wpool", bufs=1) as wp, \
             tc.tile_pool(name="hstream", bufs=8) as hsp, \
             tc.tile_pool(name="proj_ps", bufs=1, space="PSUM") as prps:
            wq_s = wp.tile([128, ND, NQH * HD], F32R)
            wk_s = wp.tile([128, ND, HD], F32R)
            wv_s = wp.tile([128, ND, HD], F32R)
            for d in range(ND):
                nc.sync.dma_start(wq_s[:, d, :], r(wq[d * 128:(d + 1) * 128, :]))
                nc.sync.dma_start(wk_s[:, d, :], r(wk[d * 128:(d + 1) * 128, :]))
                nc.sync.dma_start(wv_s[:, d, :], r(wv[d * 128:(d + 1) * 128, :]))
            for tcj in range(NTC):
                ts = slice(tcj * TC, (tcj + 1) * TC)
                pq = [prps.tile([128, TC], F32, tag=f"pq{h}", name=f"pq{h}") for h in range(NQH)]
                pk = prps.tile([128, TC], F32, tag="pk")
                pv = prps.tile([128, TC], F32, tag="pv")
                for d in range(ND):
                    hs_d = hsp.tile([128, TC], F32R)
                    nc.sync.dma_start(hs_d[:], r(hsT[d * 128:(d + 1) * 128, ts]))
                    st, sp = (d == 0), (d == ND - 1)
                    for h in range(NQH):
                        nc.tensor.matmul(pq[h][:], wq_s[:, d, h * HD:(h + 1) * HD],
                                         hs_d[:], start=st, stop=sp)
                    nc.tensor.matmul(pk[:], wk_s[:, d, :], hs_d[:], start=st, stop=sp)
                    nc.tensor.matmul(pv[:], wv_s[:, d, :], hs_d[:], start=st, stop=sp)
                for h in range(NQH):
                    nc.scalar.copy(qT[h][:, ts], pq[h][:])
                nc.scalar.copy(kT[:, R + tcj * TC: R + (tcj + 1) * TC], pk[:])
                nc.scalar.copy(vT[:, ts], pv[:])

        # ============ Phase 3: per-head RMSNorm + RoPE on q, k ============
        with tc.tile_pool(name="nrm_ps", bufs=2, space="PSUM") as nps, \
             tc.tile_pool(name="nrm_sb", bufs=2) as nsb:
            targets = [(qT[h], qw_s) for h in range(NQH)] + [(None, kw_s)]
            for xT, w in targets:
                get = (lambda a, b: xT[:, a:b]) if xT is not None else \
                      (lambda a, b: kT[:, R + a: R + b])
                for j in range(NTC):
                    a, b = j * TC, (j + 1) * TC
                    sq = nsb.tile([HD, TC], F32R, tag="sq", bufs=3)
                    nc.scalar.square(sq[:], get(a, b).bitcast(F32))
                    ssp = nps.tile([1, TC], F32, tag="ss")
                    nc.tensor.matmul(ssp[:], onesc_s[:], sq[:],
                                     start=True, stop=True)
                    srt = nsb.tile([1, TC], F32, tag="srt")
                    nc.scalar.activation(srt[:], ssp[:], ACTF.Sqrt,
                                         bias=epsc[0:1, :], scale=1.0 / HD)
                    rinv = nsb.tile([1, TC], F32R, tag="rinv")
                    with nc.allow_low_precision(reason="f32r same width as f32"):
                        nc.vector.reciprocal(rinv[:], srt[:])
                    rb = nps.tile([128, TC], F32, tag="rb")
                    nc.tensor.matmul(rb[:], onesr_s[:], rinv[:],
                                     start=True, stop=True)
                    xn = nsb.tile([HD, TC], F32R, tag="xn")
                    nc.vector.scalar_tensor_tensor(
                        xn[:], get(a, b).bitcast(F32), w[:], rb[:],
                        op0=ALU.mult, op1=ALU.mult)
                    pr = nps.tile([HD, TC], F32, tag="pr")
                    nc.tensor.matmul(pr[:], rotm_s[:], xn[:], start=True, stop=True)
                    t1 = nsb.tile([HD, TC], F32, tag="t1")
                    nc.vector.tensor_mul(t1[:], xn[:].bitcast(F32), cosT_s[:, a:b])
                    t2 = nsb.tile([HD, TC], F32, tag="t2")
                    nc.vector.tensor_mul(t2[:], pr[:], sinT_s[:, a:b])
                    nc.vector.tensor_add(get(a, b), t1[:], t2[:])

        # ============ Phase 4: transpose V to natural layout ============
        with tc.tile_pool(name="vt_ps", bufs=4, space="PSUM") as vps:
            for b in range(T // 128):
                pt = vps.tile([128, 128], F32R)
                nc.tensor.transpose(pt[:], vT[:, b * 128:(b + 1) * 128], ident_s[:])
                nc.scalar.copy(vnat[:, (b + 1) * 128:(b + 2) * 128], pt[:])

        # ================= Phase 5: attention =================
        with tc.tile_pool(name="at_ps", bufs=1, space="PSUM") as aps, \
             tc.tile_pool(name="at_acc", bufs=2, space="PSUM") as accps, \
             tc.tile_pool(name="at_sum", bufs=2, space="PSUM") as sumps, \
             tc.tile_pool(name="at_sb", bufs=4) as asb:
            for tcj in range(NTC):
                for h in range(NQH):
                    ts = slice(tcj * TC, (tcj + 1) * TC)
                    nreal = 4 * tcj + 4
                    po = accps.tile([128, TC], F32, tag="po")
                    psum = sumps.tile([1, TC], F32, tag="ps")
                    nblk = nreal + 1
                    for i in range(nblk):
                        virt = (i == 0)
                        bb = i - 1
                        rows = R if virt else 128
                        st_ = aps.tile([128, TC], F32, tag="st", bufs=3)
                        if virt:
                            lhs = kT[:, 0:R]
                        else:
                            lhs = kT[:, R + bb * 128: R + (bb + 1) * 128]
                        nc.tensor.matmul(st_[:rows, :], lhs, qT[h][:, ts],
                                         start=True, stop=True)
                        if (not virt) and bb >= 4 * tcj:
                            j = bb - 4 * tcj
                            nc.vector.tensor_add(
                                st_[:], st_[:], maskT_s[:, j * TC:(j + 1) * TC])
                        pe = asb.tile([128, TC], F32R, tag="pe", bufs=6)
                        nc.scalar.activation(pe[:rows, :], st_[:rows, :], ACTF.Exp,
                                             scale=SCALING)
                        nc.tensor.matmul(psum[:], onesc_s[:rows, :], pe[:rows, :],
                                         start=(i == 0), stop=(i == nblk - 1))
                        if virt:
                            vsl = vnat[0:R, 0:128]
                        else:
                            vsl = vnat[:, (bb + 1) * 128:(bb + 2) * 128]
                        nc.tensor.matmul(po[:], vsl, pe[:rows, :],
                                         start=(i == 0), stop=(i == nblk - 1))
                    rinv = asb.tile([1, TC], F32R, tag="arinv")
                    with nc.allow_low_precision(reason="f32r same width as f32"):
                        nc.vector.reciprocal(rinv[:], psum[:])
                    rb = aps.tile([128, TC], F32, tag="arb", bufs=1)
                    nc.tensor.matmul(rb[:], onesr_s[:], rinv[:], start=True, stop=True)
                    rbs = asb.tile([128, TC], F32, tag="rbs")
                    nc.scalar.copy(rbs[:], rb[:])
                    nc.vector.tensor_mul(oT[h][:, ts], po[:], rbs[:])

        # ================= Phase 6: output projection =================
        with tc.tile_pool(name="op_ps", bufs=4, space="PSUM") as ops, \
             tc.tile_pool(name="wo_sb", bufs=2) as wosb, \
             tc.tile_pool(name="out_sb", bufs=4) as outsb:
            for j2 in range(D // TC):
                wo_t = [wosb.tile([128, TC], F32R, tag=f"wo{h}", name=f"wo{h}")
                        for h in range(NQH)]
                for h in range(NQH):
                    nc.sync.dma_start(
                        wo_t[h][:],
                        r(wo[h * HD:(h + 1) * HD, j2 * TC:(j2 + 1) * TC]))
                for tt in range(T // 128):
                    po = ops.tile([128, TC], F32, tag="opo")
                    for h in range(NQH):
                        nc.tensor.matmul(po[:], oT[h][:, tt * 128:(tt + 1) * 128],
                                         wo_t[h][:], start=(h == 0), stop=(h == NQH - 1))
                    ob = outsb.tile([128, TC], F32, tag="ob")
                    nc.scalar.copy(ob[:], po[:])
                    nc.sync.dma_start(
                        out[tt * 128:(tt + 1) * 128, j2 * TC:(j2 + 1) * TC], ob[:])

    nc.compile()
    return nc


_NC_CACHE = {}


def _get_nc():
    if "nc" not in _NC_CACHE:
        _NC_CACHE["nc"] = build_nc()
    return _NC_CACHE["nc"]


def kernel(**inputs) -> np.ndarray:
    f = lambda k: np.asarray(inputs[k], np.float32)
    hs = f("hidden_states")[0]            # (T, D)
    vk = f("virtual_keys")[0]             # (HKV, R, HD)
    vv = f("virtual_values")[0]
    Wq, Wk, Wv, Wo = f("Wq"), f("Wk"), f("Wv"), f("Wo")
    qnw, knw = f("q_norm_w"), f("k_norm_w")
    lkA, lkB = f("lora_k_A"), f("lora_k_B")
    lvA, lvB = f("lora_v_A"), f("lora_v_B")
    sk = np.float32(np.asarray(inputs["scale_k"]))
    sv = np.float32(np.asarray(inputs["scale_v"]))
    am = f("attention_mask")              # (1,1,T,T)
    cos, sin = f("cos"), f("sin")         # (T, HD)

    hsT = np.ascontiguousarray(hs.T)
    cosT = np.ascontiguousarray(cos.T)
    sinT = np.ascontiguousarray(sin.T)
    # diagonal causal mask blocks, transposed: block j = am[0,0,0:512,128j:+128].T
    maskT = np.ascontiguousarray(
        np.concatenate([am[0, 0, 0:TC, 128 * j:128 * (j + 1)].T for j in range(4)],
                       axis=1))
    rotm = np.zeros((HD, HD), np.float32)
    for dd in range(64):
        rotm[dd + 64, dd] = -1.0          # rot[d] = -x[d+64], d<64
        rotm[dd, dd + 64] = 1.0           # rot[d] = +x[d-64], d>=64
    ident = np.eye(128, dtype=np.float32)
    onesc = np.ones((128, 1), np.float32)
    onesr = np.ones((1, 128), np.float32)
    lkBs = np.ascontiguousarray(lkB * sk)
    lvBs = np.ascontiguousarray(lvB * sv)

    in_maps = []
    for m in range(8):
        in_maps.append({
            "hsT": hsT,
            "wq": np.ascontiguousarray(Wq[:, 512 * m:512 * (m + 1)]),
            "wk": np.ascontiguousarray(Wk[:, 128 * m:128 * (m + 1)]),
            "wv": np.ascontiguousarray(Wv[:, 128 * m:128 * (m + 1)]),
            "wo": np.ascontiguousarray(Wo[512 * m:512 * (m + 1), :]),
            "vkT": np.ascontiguousarray(vk[m].T),
            "vvT": np.ascontiguousarray(vv[m].T),
            "lkA": lkA, "lkB": lkBs, "lvA": lvA, "lvB": lvBs,
            "qw": np.ascontiguousarray(qnw[:, None]),
            "kw": np.ascontiguousarray(knw[:, None]),
            "cosT": cosT, "sinT": sinT, "maskT": maskT,
            "rotm": rotm, "ident": ident, "onesc": onesc, "onesr": onesr,
        })

    nc = _get_nc()
    res = run_bass_kernel_spmd(nc, in_maps, core_ids=list(range(8)))
    acc = res.results[0]["out"].astype(np.float32)
    for m in range(1, 8):
        acc = acc + res.results[m]["out"]
    return acc[None]  # (1, T, D)



# revision 10
# speedup vs baseline: 1.2968x; 1.2968x over previous
"""Trainium2 Bass kernel for KVAdapterInjector (Qwen3-style GQA attention with
LoRA-adapted virtual KV prefix).

Sharding: tensor-parallel over heads across 8 cores. Core m gets KV head m and
Q heads 4m..4m+3. Wq/Wk/Wv sharded on output dim, Wo on input dim; partial
outputs (bf16) summed on host.

v2 design notes (cost-model driven):
- All heavy matmuls in bf16 (1.0 cycles/row, immune to the fp32r ap<256
  penalty). PSUM accumulation stays fp32. Measured end-to-end bf16 error
  ~5.5e-3 (budget 2e-2). fp8 was measured at 2.7-5e-2 per stage: rejected.
- PE-row accounting puts the tensor engine at ~370us; all other engines are
  kept under ~150us: softmax denominators stay as ones-matmuls on PE, but
  rms-norm sum/broadcast use gpsimd partition_all_reduce/broadcast (Pool),
  rsqrt = exp(-0.5*ln(x)) on Act (single activation table: ln+exp+square),
  mask-adds and PSUM drains ride Pool, rope elementwise rides DVE in bf16
  (2x mode).
- Causal diagonal blocks are trimmed: block j of a 512-query chunk only
  computes queries >= 128*j, with a constant [128,128] triangular mask tile.
- Chunk-pipelined: proj(c) -> norm/rope(c) -> attention(c) -> outproj(c),
  with PSUM pools sized to exactly 8 banks so phases from adjacent chunks
  overlap across engines.
"""
import sys

sys.path.insert(0, "/opt/trn_rl_repo")

import numpy as np
import ml_dtypes

import concourse.bass as bass
import concourse.mybir as mybir
import concourse.tile as tile
from concourse import bacc
from concourse import bass_isa
from concourse.bass_utils import run_bass_kernel_spmd

F32 = mybir.dt.float32
F32R = mybir.dt.float32r
BF16 = mybir.dt.bfloat16
AX = mybir.AxisListType
ALU = mybir.AluOpType
ACTF = mybir.ActivationFunctionType
RED = bass_isa.ReduceOp

T = 2048
D = 4096
HD = 128
NQH = 4          # q heads per core
R = 64           # virtual tokens
RANK = 16
EPS = 1e-6
SCALING = HD ** -0.5
NTC = 4          # T chunks of 512
TC = 512
ND = D // 128    # 32 contraction tiles
NKB = T // 128   # 16 key blocks (real)


def build_nc():
    nc = bacc.Bacc(None, target_bir_lowering=False, debug=False)

    # ---- DRAM I/O (bf16 activations/weights prepared on host) ----
    hsT = nc.dram_tensor("hsT", (D, T), BF16, kind="ExternalInput")
    wq = nc.dram_tensor("wq", (D, NQH * HD), BF16, kind="ExternalInput")
    wk = nc.dram_tensor("wk", (D, HD), BF16, kind="ExternalInput")
    wv = nc.dram_tensor("wv", (D, HD), BF16, kind="ExternalInput")
    wo = nc.dram_tensor("wo", (NQH * HD, D), BF16, kind="ExternalInput")
    cwq = nc.dram_tensor("cwq", (HD, T), BF16, kind="ExternalInput")
    swq = nc.dram_tensor("swq", (HD, T), BF16, kind="ExternalInput")
    cwk = nc.dram_tensor("cwk", (HD, T), BF16, kind="ExternalInput")
    swk = nc.dram_tensor("swk", (HD, T), BF16, kind="ExternalInput")
    masktri = nc.dram_tensor("masktri", (128, 128), F32, kind="ExternalInput")
    vkT = nc.dram_tensor("vkT", (HD, R), F32, kind="ExternalInput")
    vvT = nc.dram_tensor("vvT", (HD, R), F32, kind="ExternalInput")
    lkA = nc.dram_tensor("lkA", (HD, RANK), F32, kind="ExternalInput")
    lkB = nc.dram_tensor("lkB", (RANK, HD), F32, kind="ExternalInput")  # pre-scaled
    lvA = nc.dram_tensor("lvA", (HD, RANK), F32, kind="ExternalInput")
    lvB = nc.dram_tensor("lvB", (RANK, HD), F32, kind="ExternalInput")  # pre-scaled
    ident = nc.dram_tensor("ident", (128, 128), F32, kind="ExternalInput")
    out = nc.dram_tensor("out", (T, D), BF16, kind="ExternalOutput")

    r = lambda ap: ap.bitcast(F32R)

    from contextlib import ExitStack
    with tile.TileContext(nc) as tc, ExitStack() as est:
        cp = est.enter_context(tc.tile_pool(name="consts", bufs=1))
        pp = est.enter_context(tc.tile_pool(name="persist", bufs=1))

        # ---- small consts ----
        onesb = cp.tile([128, 1], BF16)
        nc.vector.memset(onesb[:], 1.0)
        epsc = cp.tile([128, 1], F32)
        nc.vector.memset(epsc[:], EPS)
        zeroc = cp.tile([128, 1], F32)
        nc.vector.memset(zeroc[:], 0.0)
        mask_s = cp.tile([128, 128], F32)
        nc.sync.dma_start(mask_s[:], masktri[:])

        # ---- persistent activations ----
        # qT[h]: rope'd queries, [HD, T] bf16; aliased as oT (attention output)
        qT = [pp.tile([HD, T], BF16, tag=f"qT{h}", name=f"qT{h}") for h in range(NQH)]
        oT = qT
        kT = pp.tile([HD, R + T], BF16)           # cols 0:64 = adapted virtual keys
        vnat = pp.tile([128, NKB + 1, 128], BF16)  # block 0 = virtual values (rows 0:64)

        # ---- rope/norm consts (weighted cos/sin) ----
        cwq_s = cp.tile([HD, T], BF16)
        swq_s = cp.tile([HD, T], BF16)
        cwk_s = cp.tile([HD, T], BF16)
        swk_s = cp.tile([HD, T], BF16)

        # ---- weights in SBUF ----
        wqk_s = cp.tile([128, ND, NQH * HD + HD], BF16)   # q cols 0:512, k cols 512:640
        wv_s = cp.tile([128, ND, HD], BF16)
        wo_s = cp.tile([128, NQH, D], BF16)

        # ================= Phase 0: LoRA-adapt virtual KV (tiny) =================
        with tc.tile_pool(name="lora_ps", bufs=1, space="PSUM") as lps, \
             tc.tile_pool(name="lora_sb", bufs=1) as lsb:
            vkT_s = lsb.tile([HD, R], F32R)
            vvT_s = lsb.tile([HD, R], F32R)
            lkA_s = lsb.tile([HD, RANK], F32R)
            lkB_s = lsb.tile([RANK, HD], F32R)
            lvA_s = lsb.tile([HD, RANK], F32R)
            lvB_s = lsb.tile([RANK, HD], F32R)
            ident_s = lsb.tile([128, 128], F32R)
            nc.sync.dma_start(vkT_s[:], r(vkT[:]))
            nc.sync.dma_start(vvT_s[:], r(vvT[:]))
            nc.sync.dma_start(lkA_s[:], r(lkA[:]))
            nc.sync.dma_start(lkB_s[:], r(lkB[:]))
            nc.sync.dma_start(lvA_s[:], r(lvA[:]))
            nc.sync.dma_start(lvB_s[:], r(lvB[:]))
            nc.sync.dma_start(ident_s[:], r(ident[:]))
            # keys: kT[:, 0:64] = vkT + Bk^T Ak^T vkT  (Bk pre-scaled)
            t1 = lps.tile([RANK, R], F32, tag="l1")
            nc.tensor.matmul(t1[:], lkA_s[:], vkT_s[:], start=True, stop=True)
            t1s = lsb.tile([RANK, R], F32R)
            nc.scalar.copy(t1s[:], t1[:])
            t2 = lps.tile([HD, R], F32, tag="l2")
            nc.tensor.matmul(t2[:], lkB_s[:], t1s[:], start=True, stop=True)
            nc.vector.tensor_add(kT[:, 0:R], vkT_s[:].bitcast(F32), t2[:])
            # values
            u1 = lps.tile([RANK, R], F32, tag="l1")
            nc.tensor.matmul(u1[:], lvA_s[:], vvT_s[:], start=True, stop=True)
            u1s = lsb.tile([RANK, R], F32R)
            nc.scalar.copy(u1s[:], u1[:])
            u2 = lps.tile([HD, R], F32, tag="l2")
            nc.tensor.matmul(u2[:], lvB_s[:], u1s[:], start=True, stop=True)
            vvirt = lsb.tile([HD, R], F32R)
            with nc.allow_low_precision(reason="f32r same width as f32"):
                nc.vector.tensor_add(vvirt[:], vvT_s[:].bitcast(F32), u2[:])
            # transpose virtual values to natural layout -> vnat[0:64, 0, :]
            vtp = lps.tile([R, HD], F32R, tag="l3")
            nc.tensor.transpose(vtp[:], vvirt[:], ident_s[:])
            nc.gpsimd.tensor_copy(vnat[0:R, 0, :], vtp[:].bitcast(F32))

        # ---- weight / rope-const loads (after lora pool closes) ----
        nc.sync.dma_start(cwq_s[:], cwq[:])
        nc.sync.dma_start(swq_s[:], swq[:])
        nc.sync.dma_start(cwk_s[:], cwk[:])
        nc.sync.dma_start(swk_s[:], swk[:])
        # batched weight loads: one 3D-AP DMA per tensor (partition-major view)
        nc.sync.dma_start(wqk_s[:, :, 0:NQH * HD],
                          wq[:, :].rearrange("(n p) c -> p n c", p=128))
        nc.sync.dma_start(wqk_s[:, :, NQH * HD:],
                          wk[:, :].rearrange("(n p) c -> p n c", p=128))
        nc.sync.dma_start(wv_s[:],
                          wv[:, :].rearrange("(n p) c -> p n c", p=128))
        nc.sync.dma_start(wo_s[:],
                          wo[:, :].rearrange("(n p) c -> p n c", p=128))

        # ================= main chunk pipeline =================
        with tc.tile_pool(name="proj_ps", bufs=2, space="PSUM") as prps, \
             tc.tile_pool(name="mm_ps", bufs=3, space="PSUM") as mmps, \
             tc.tile_pool(name="den_ps", bufs=1, space="PSUM") as dnps, \
             tc.tile_pool(name="b2k_ps", bufs=2, space="PSUM") as b2ps, \
             tc.tile_pool(name="hs_sb", bufs=1) as hsb, \
             tc.tile_pool(name="nrm_sb", bufs=2) as nsb, \
             tc.tile_pool(name="pe_sb", bufs=6) as peb, \
             tc.tile_pool(name="at_sb", bufs=2) as asb, \
             tc.tile_pool(name="ob_sb", bufs=2) as obb:
            hs_tiles = {0: hsb.tile([128, ND, TC], BF16, tag="hs", name="hs0")}
            nc.sync.dma_start(hs_tiles[0][:],
                              hsT[:, 0:TC].rearrange("(n p) t -> p n t", p=128))
            for c in range(NTC):
                ts = slice(c * TC, (c + 1) * TC)
                hs_c = hs_tiles.pop(c)

                # ---- projections: 5 passes (q0..q3, k), each one accumulator ----
                for p in range(NQH + 1):
                    pacc = prps.tile([128, TC], F32, tag="pacc")
                    wslice = wqk_s[:, :, p * HD:(p + 1) * HD]
                    for d in range(ND):
                        nc.tensor.matmul(pacc[:], wslice[:, d, :], hs_c[:, d, :],
                                         start=(d == 0), stop=(d == ND - 1))
                    # ---- rms-norm + rope on this pass's PSUM ----
                    isq = p < NQH
                    cw = cwq_s if isq else cwk_s
                    sw = swq_s if isq else swk_s
                    dst = qT[p][:, ts] if isq else kT[:, R + c * TC: R + (c + 1) * TC]
                    sq = nsb.tile([HD, TC], BF16, tag="sq")
                    nc.gpsimd.tensor_mul(sq[:], pacc[:], pacc[:])
                    ssum = nsb.tile([HD, TC], F32, tag="ssum")
                    nc.gpsimd.partition_all_reduce(ssum[:], sq[:], channels=128,
                                                   reduce_op=RED.add)
                    lns = nsb.tile([HD, TC], F32, tag="lns")
                    nc.scalar.activation(lns[:], ssum[:], ACTF.Ln,
                                         scale=1.0 / HD, bias=epsc[:])
                    rinv = nsb.tile([HD, TC], F32, tag="rinv")
                    nc.scalar.activation(rinv[:], lns[:], ACTF.Exp, scale=-0.5,
                                         bias=zeroc[:])
                    xn = nsb.tile([HD, TC], BF16, tag="xn")
                    nc.vector.tensor_mul(xn[:], pacc[:], rinv[:])
                    t1 = nsb.tile([HD, TC], BF16, tag="t1")
                    nc.vector.tensor_mul(t1[:], xn[:], cw[:, ts])
                    t2 = nsb.tile([HD, TC], BF16, tag="t2")
                    nc.vector.tensor_mul(t2[0:64, :], xn[64:128, :], sw[0:64, ts])
                    nc.vector.tensor_mul(t2[64:128, :], xn[0:64, :], sw[64:128, ts])
                    nc.vector.tensor_add(dst, t1[:], t2[:])

                # ---- V in natural layout: stationary = hs t-slices ----
                vacc = b2ps.tile([128, 4, 128], F32, tag="b2k")
                for tt in range(4):
                    for d in range(ND):
                        nc.tensor.matmul(vacc[:, tt, :],
                                         hs_c[:, d, tt * 128:(tt + 1) * 128],
                                         wv_s[:, d, :],
                                         start=(d == 0), stop=(d == ND - 1))
                nc.gpsimd.tensor_copy(vnat[:, 1 + 4 * c: 5 + 4 * c, :], vacc[:])

                # prefetch next chunk's hidden states (after last hs_c reader issued)
                if c + 1 < NTC:
                    hs_tiles[c + 1] = hsb.tile([128, ND, TC], BF16, tag="hs",
                                                name=f"hs{c + 1}")
                    nc.sync.dma_start(
                        hs_tiles[c + 1][:],
                        hsT[:, (c + 1) * TC:(c + 2) * TC].rearrange(
                            "(n p) t -> p n t", p=128))

                # ---- attention for this chunk's queries ----
                for h in range(NQH):
                    den = dnps.tile([1, TC], F32, tag="den")
                    po = b2ps.tile([128, TC], F32, tag="b2k")
                    # virtual block (full width)
                    st_ = mmps.tile([128, TC], F32, tag="mm")
                    nc.tensor.matmul(st_[0:R, :], kT[:, 0:R], qT[h][:, ts],
                                     start=True, stop=True)
                    pe = peb.tile([128, TC], BF16, tag="pe")
                    nc.scalar.activation(pe[0:R, :], st_[0:R, :], ACTF.Exp,
                                         scale=SCALING, bias=zeroc[0:R, :])
                    nc.tensor.matmul(den[:], onesb[0:R, :], pe[0:R, :],
                                     start=True, stop=False)
                    nc.tensor.matmul(po[:], vnat[0:R, 0, :], pe[0:R, :],
                                     start=True, stop=False)
                    # full (past) key blocks
                    for bb in range(4 * c):
                        st_ = mmps.tile([128, TC], F32, tag="mm")
                        nc.tensor.matmul(st_[:], kT[:, R + bb * 128: R + (bb + 1) * 128],
                                         qT[h][:, ts], start=True, stop=True)
                        pe = peb.tile([128, TC], BF16, tag="pe")
                        nc.scalar.activation(pe[:], st_[:], ACTF.Exp,
                                             scale=SCALING, bias=zeroc[:])
                        nc.tensor.matmul(den[:], onesb[:], pe[:],
                                         start=False, stop=False)
                        nc.tensor.matmul(po[:], vnat[:, 1 + bb, :], pe[:],
                                         start=False, stop=False)
                    # diagonal blocks j=0..3: queries >= 128*j only
                    for j in range(4):
                        bb = 4 * c + j
                        W = TC - 128 * j
                        qs = slice(c * TC + 128 * j, (c + 1) * TC)
                        st_ = mmps.tile([128, TC], F32, tag="mm")
                        nc.tensor.matmul(st_[:, 0:W],
                                         kT[:, R + bb * 128: R + (bb + 1) * 128],
                                         qT[h][:, qs], start=True, stop=True)
                        # triangular mask on the first 128 cols of this region
                        nc.gpsimd.tensor_add(st_[:, 0:128], st_[:, 0:128], mask_s[:])
                        pe = peb.tile([128, TC], BF16, tag="pe")
                        nc.scalar.activation(pe[:, 0:W], st_[:, 0:W], ACTF.Exp,
                                             scale=SCALING, bias=zeroc[:])
                        last = (j == 3)
                        nc.tensor.matmul(den[:, 128 * j:], onesb[:], pe[:, 0:W],
                                         start=False, stop=last)
                        nc.tensor.matmul(po[:, 128 * j:], vnat[:, 1 + bb, :], pe[:, 0:W],
                                         start=False, stop=last)
                    # normalize: oT[h][:, ts] = po * (1/den[h]) broadcast
                    rc = asb.tile([1, TC], F32, tag="rc")
                    nc.vector.reciprocal(rc[:], den[:])
                    rb = asb.tile([128, TC], F32, tag="rb")
                    nc.gpsimd.partition_broadcast(rb[:], rc[:], channels=128)
                    nc.vector.tensor_mul(oT[h][:, ts], po[:], rb[:])

                # ---- output projection for this chunk's 4 t-tiles ----
                for tt in range(4 * c, 4 * c + 4):
                    ob = obb.tile([128, 8, TC], BF16, tag="ob")
                    for j2 in range(D // TC):
                        po2 = mmps.tile([128, TC], F32, tag="mm")
                        for h in range(NQH):
                            nc.tensor.matmul(po2[:], oT[h][:, tt * 128:(tt + 1) * 128],
                                             wo_s[:, h, j2 * TC:(j2 + 1) * TC],
                                             start=(h == 0), stop=(h == NQH - 1))
                        nc.gpsimd.tensor_copy(ob[:, j2, :], po2[:])
                    eng = nc.sync if tt % 2 == 0 else nc.scalar
                    eng.dma_start(out[tt * 128:(tt + 1) * 128, :], ob[:])

    nc.compile()
    return nc


_NC_CACHE = {}


def _get_nc():
    if "nc" not in _NC_CACHE:
        _NC_CACHE["nc"] = build_nc()
    return _NC_CACHE["nc"]


def _bf(x):
    return np.ascontiguousarray(x.astype(ml_dtypes.bfloat16))


def kernel(**inputs) -> np.ndarray:
    f = lambda k: np.asarray(inputs[k], np.float32)
    hs = f("hidden_states")[0]            # (T, D)
    vk = f("virtual_keys")[0]             # (HKV, R, HD)
    vv = f("virtual_values")[0]
    Wq, Wk, Wv, Wo = f("Wq"), f("Wk"), f("Wv"), f("Wo")
    qnw, knw = f("q_norm_w"), f("k_norm_w")
    lkA, lkB = f("lora_k_A"), f("lora_k_B")
    lvA, lvB = f("lora_v_A"), f("lora_v_B")
    sk = np.float32(np.asarray(inputs["scale_k"]))
    sv = np.float32(np.asarray(inputs["scale_v"]))
    cos, sin = f("cos"), f("sin")         # (T, HD)

    hsT = _bf(hs.T)
    # weighted cos/sin for fused (rms*w) + rope:
    #   cw[d,t] = w[d]*cos[t,d]
    #   sw[d,t] = -w[d+64]*sin[t,d]  (d<64);  w[d-64]*sin[t,d]  (d>=64)
    def cw_sw(w):
        cw = (cos.T * w[:, None]).astype(np.float32)
        sw = np.empty((HD, T), np.float32)
        sw[0:64] = -w[64:128, None] * sin.T[0:64]
        sw[64:128] = w[0:64, None] * sin.T[64:128]
        return _bf(cw), _bf(sw)
    cwqh, swqh = cw_sw(qnw)
    cwkh, swkh = cw_sw(knw)
    # constant [128,128] triangular mask: allowed k<=q, else -1e30
    idx = np.arange(128)
    masktri = np.where(idx[:, None] <= idx[None, :], 0.0, -1e30).astype(np.float32)
    ident = np.eye(128, dtype=np.float32)
    lkBs = np.ascontiguousarray(lkB * sk)
    lvBs = np.ascontiguousarray(lvB * sv)

    in_maps = []
    for m in range(8):
        in_maps.append({
            "hsT": hsT,
            "wq": _bf(Wq[:, 512 * m:512 * (m + 1)]),
            "wk": _bf(Wk[:, 128 * m:128 * (m + 1)]),
            "wv": _bf(Wv[:, 128 * m:128 * (m + 1)]),
            "wo": _bf(Wo[512 * m:512 * (m + 1), :]),
            "cwq": cwqh, "swq": swqh, "cwk": cwkh, "swk": swkh,
            "masktri": masktri,
            "vkT": np.ascontiguousarray(vk[m].T),
            "vvT": np.ascontiguousarray(vv[m].T),
            "lkA": lkA, "lkB": lkBs, "lvA": lvA, "lvB": lvBs,
            "ident": ident,
        })

    nc = _get_nc()
    res = run_bass_kernel_spmd(nc, in_maps, core_ids=list(range(8)))
    acc = res.results[0]["out"].astype(np.float32)
    for m in range(1, 8):
        acc = acc + res.results[m]["out"].astype(np.float32)
    return acc[None]  # (1, T, D)
